# revision 1
# baseline (speedup 1.0000x reference)
"""Trainium2 Bass kernel for the FFT-contrastive loss (nn_FCR_41704132444314).

Math (reference):
    f  = fft2(x) / (||f||_C + 1e-8) * 0.01          per-sample channel-normalized spectrum
    d_ap[b]   = mean |af_b - pf_b|                   (complex magnitude, mean over C,H,W)
    d_an[b,k] = mean |af_b - nf_{neg_idx[b,k]}|
    out = sum_{b,k} d_ap[b] / (d_an[b,k] + 1e-7) / (K*B)

Device strategy (8 cores, data-parallel over batch, negatives gathered on host):
  - 2D FFT as DFT-by-matmul: Y = F @ X @ F with F the 256-point DFT matrix
    (stage A: U = F@X, PE transpose of U, stage B: Y = U@F), all in bf16 with
    f32 PSUM accumulation.
  - Hermitian symmetry of real-input FFTs: only rows k1=1..128 are computed on
    device (weights 2 for k1=1..127, 1 for k1=128, applied via the per-partition
    scale of the fused sqrt); the k1=0 row is reconstructed on host with a tiny
    1-D numpy FFT of the column sums.
  - Per sample the device emits 3 weighted row-sums (pairs ap/an1/an2) per
    k1-partition; host adds the row-0 terms and forms the final scalar.
"""

import sys

sys.path.insert(0, "/opt/trn_rl_repo")

import numpy as np
import ml_dtypes

bf16 = ml_dtypes.bfloat16

B, C, H, W = 64, 3, 256, 256
K = 2
N_CORES = 8
SPC = B // N_CORES  # samples per core
BF = None  # mybir.dt.bfloat16, set lazily
_PROGRAM = None  # cached (nc, const_inputs)


def _build_program(spc=SPC):
    import concourse.bacc as bacc
    import concourse.mybir as mybir
    from concourse import tile

    f32 = mybir.dt.float32
    bft = mybir.dt.bfloat16

    nc = bacc.Bacc(trn_type="TRN2", target_bir_lowering=False, debug=False)

    a_d = nc.dram_tensor("a_in", [spc, C, H, W], bft, kind="ExternalInput")
    p_d = nc.dram_tensor("p_in", [spc, C, H, W], bft, kind="ExternalInput")
    n_d = nc.dram_tensor("n_in", [spc * K, C, H, W], bft, kind="ExternalInput")
    fr_d = nc.dram_tensor("fr", [256, 256], bft, kind="ExternalInput")
    fi_d = nc.dram_tensor("fi", [256, 256], bft, kind="ExternalInput")
    frfi_d = nc.dram_tensor("frfi", [256, 512], bft, kind="ExternalInput")
    finfr_d = nc.dram_tensor("finfr", [256, 512], bft, kind="ExternalInput")
    id_d = nc.dram_tensor("ident", [128, 128], bft, kind="ExternalInput")
    w2_d = nc.dram_tensor("w2", [128, 1], f32, kind="ExternalInput")
    rs_d = nc.dram_tensor("rs_out", [128, spc, 3], f32, kind="ExternalOutput")

    from contextlib import ExitStack

    with tile.TileContext(nc) as tc, ExitStack() as es:
        cp = es.enter_context(tc.tile_pool(name="consts", bufs=1))
        # Stage-A weights: h = 2p + j interleave (matches the X load layout)
        cFrA = cp.tile([128, 2, 256], bft, name="cFrA")
        cFiA = cp.tile([128, 2, 256], bft, name="cFiA")
        # Stage-B rhs: w = m*128 + q block split (matches the transpose layout),
        # with [Fr|Fi] and [-Fi|Fr] concatenated so each channel's (Yr|Yi) is a
        # single PSUM accumulation group.
        cFrFiB = cp.tile([128, 2, 512], bft, name="cFrFiB")
        cFinFrB = cp.tile([128, 2, 512], bft, name="cFinFrB")
        cId = cp.tile([128, 128], bft, name="cId")
        cW2 = cp.tile([128, 1], f32, name="cW2")
        rs_all = cp.tile([128, spc * 3], f32, name="rs_all")

        nc.sync.dma_start(out=cFrA[:], in_=fr_d.ap().rearrange("(p j) k -> p j k", j=2))
        nc.sync.dma_start(out=cFiA[:], in_=fi_d.ap().rearrange("(p j) k -> p j k", j=2))
        nc.sync.dma_start(out=cFrFiB[:], in_=frfi_d.ap().rearrange("(m q) k -> q m k", q=128))
        nc.sync.dma_start(out=cFinFrB[:], in_=finfr_d.ap().rearrange("(m q) k -> q m k", q=128))
        nc.sync.dma_start(out=cId[:], in_=id_d.ap())
        nc.sync.dma_start(out=cW2[:], in_=w2_d.ap())

        xp = es.enter_context(tc.tile_pool(name="xp", bufs=8))
        usbp = es.enter_context(tc.tile_pool(name="usbp", bufs=5))
        utp = es.enter_context(tc.tile_pool(name="utp", bufs=5))
        ypkp = es.enter_context(tc.tile_pool(name="ypkp", bufs=6))
        fscp = es.enter_context(tc.tile_pool(name="fscp", bufs=8))
        scrp = es.enter_context(tc.tile_pool(name="scrp", bufs=5))
        pU = es.enter_context(tc.tile_pool(name="pU", bufs=1, space="PSUM"))
        pT = es.enter_context(tc.tile_pool(name="pT", bufs=2, space="PSUM"))
        pY = es.enter_context(tc.tile_pool(name="pY", bufs=2, space="PSUM"))

        def fft_image(src_ap):
            """src_ap: DRAM [C,H,W] bf16. Returns fsc tile [128, 2, 3, 256] bf16:
            channel-normalized spectrum rows k1=1..128 (partition = k1-1)."""
            X = xp.tile([128, 3, 2, 256], bft, name="X", tag="X")
            for c in range(3):
                eng = nc.sync if c != 1 else nc.scalar
                eng.dma_start(
                    out=X[:, c, :, :],
                    in_=src_ap[c].rearrange("(p j) w -> p j w", j=2),
                )
            # ---- stage A: U = F[:,1:129].T-ish @ X  (rows k1=1..128)
            Ur = pU.tile([128, 3, 256], mybir.dt.float32, name="Ur", tag="Ur")
            Ui = pU.tile([128, 3, 256], mybir.dt.float32, name="Ui", tag="Ui")
            # j-major so both matmuls sharing one weight block are adjacent
            for Upsum, cFA in ((Ur, cFrA), (Ui, cFiA)):
                for j in range(2):
                    nc.tensor.matmul(
                        Upsum[:, 0:2, :], cFA[:, j, 1:129], X[:, 0:2, j, :],
                        start=(j == 0), stop=(j == 1),
                    )
                    nc.tensor.matmul(
                        Upsum[:, 2, :], cFA[:, j, 1:129], X[:, 2, j, :],
                        start=(j == 0), stop=(j == 1),
                    )
            Ursb = usbp.tile([128, 3, 256], bft, name="Ursb", tag="Ursb")
            Uisb = usbp.tile([128, 3, 256], bft, name="Uisb", tag="Uisb")
            nc.scalar.copy(Ursb[:], Ur[:])
            nc.scalar.copy(Uisb[:], Ui[:])
            # ---- PE transposes: UT[q, m, c, k1] = U[k1, w=m*128+q]
            UrT = utp.tile([128, 2, 3, 128], bft, name="UrT", tag="UrT")
            UiT = utp.tile([128, 2, 3, 128], bft, name="UiT", tag="UiT")
            for ui, (Usb, UT) in enumerate(((Ursb, UrT), (Uisb, UiT))):
                Tp = pT.tile([128, 2, 3, 128], bft, name="Tp", tag="Tp")
                for m in range(2):
                    for c in range(3):
                        nc.tensor.transpose(
                            Tp[:, m, c, :], Usb[:, c, m * 128:(m + 1) * 128], cId[:]
                        )
                if ui == 0:
                    nc.vector.tensor_copy(UT[:], Tp[:])
                else:
                    nc.scalar.copy(UT[:], Tp[:])
            # ---- stage B: Y = U @ F  (per channel; LDW shared between r/i pairs)
            ypk = ypkp.tile([128, 2, 3, 256], bft, name="ypk", tag="ypk")
            for c in range(3):
                Yri = pY.tile([128, 2, 256], mybir.dt.float32, name="Yri", tag="Yri")
                mm = nc.tensor.matmul
                mm(Yri[:], UrT[:, 0, c, :], cFrFiB[:, 0, :], start=True, stop=False)
                mm(Yri[:], UrT[:, 1, c, :], cFrFiB[:, 1, :], start=False, stop=False)
                mm(Yri[:], UiT[:, 0, c, :], cFinFrB[:, 0, :], start=False, stop=False)
                mm(Yri[:], UiT[:, 1, c, :], cFinFrB[:, 1, :], start=False, stop=True)
                nc.scalar.copy(ypk[:, :, c, :], Yri[:])
            # ---- channel norm -> 1/||.||  -> scaled features
            SQ = scrp.tile([128, 2, 3, 256], bft, name="SQ", tag="SQ")
            nc.vector.tensor_mul(SQ[:], ypk[:], ypk[:])
            s3 = scrp.tile([128, 3, 256], bft, name="s3", tag="s3")
            nc.vector.tensor_add(s3[:], SQ[:, 0, :, :], SQ[:, 1, :, :])
            s_ = scrp.tile([128, 256], bft, name="s_", tag="s_")
            nc.vector.tensor_add(s_[:], s3[:, 0, :], s3[:, 1, :])
            nc.vector.tensor_add(s_[:], s_[:], s3[:, 2, :])
            sn = scrp.tile([128, 256], mybir.dt.float32, name="sn", tag="sn", bufs=3)
            nc.scalar.sqrt(sn[:], s_[:])
            m_ = scrp.tile([128, 256], mybir.dt.float32, name="m_", tag="m_", bufs=3)
            nc.vector.reciprocal_approx_fast(m_[:], sn[:])
            mb = scrp.tile([128, 256], bft, name="mb", tag="mb")
            nc.vector.tensor_copy(mb[:], m_[:])
            fsc = fscp.tile([128, 2, 3, 256], bft, name="fsc", tag="fsc")
            m_bc = mb[:, None, :].broadcast_to([128, 6, 256]).rearrange(
                "p (a b) w -> p a b w", a=2
            )
            nc.vector.tensor_mul(fsc[:], ypk[:], m_bc)
            return fsc

        for s in range(spc):
            fa = fft_image(a_d.ap()[s])
            fp = fft_image(p_d.ap()[s])
            fn1 = fft_image(n_d.ap()[2 * s])
            fn2 = fft_image(n_d.ap()[2 * s + 1])
            for pair, fx in enumerate((fp, fn1, fn2)):
                d_ = scrp.tile([128, 2, 3, 256], bft, name="d_", tag="d_")
                nc.vector.tensor_sub(d_[:], fa[:], fx[:])
                SQd = scrp.tile([128, 2, 3, 256], bft, name="SQd", tag="SQd")
                nc.vector.tensor_mul(SQd[:], d_[:], d_[:])
                msq = scrp.tile([128, 3, 256], bft, name="msq", tag="msq")
                nc.vector.tensor_add(msq[:], SQd[:, 0, :, :], SQd[:, 1, :, :])
                mag = scrp.tile([128, 3, 256], bft, name="mag", tag="mag", bufs=2)
                nc.scalar.activation(
                    mag[:], msq[:], mybir.ActivationFunctionType.Sqrt,
                    scale=cW2[:], accum_out=rs_all[:, 3 * s + pair:3 * s + pair + 1],
                )
        nc.sync.dma_start(
            out=rs_d.ap(), in_=rs_all[:].rearrange("p (s q) -> p s q", q=3)
        )

    nc.compile()
    return nc


def _get_program():
    global _PROGRAM
    if _PROGRAM is None:
        _PROGRAM = _build_program()
    return _PROGRAM


def _const_inputs():
    k = np.arange(256)
    ang = -2.0 * np.pi * np.outer(k, k) / 256.0
    Fr = np.cos(ang).astype(np.float32)
    Fi = np.sin(ang).astype(np.float32)
    w2 = np.full((128, 1), 4.0, np.float32)
    w2[127] = 1.0  # k1 = 128 appears once; k1 = 1..127 twice (weight^2 inside sqrt)
    return {
        "fr": Fr.astype(bf16),
        "fi": Fi.astype(bf16),
        "frfi": np.concatenate([Fr, Fi], axis=1).astype(bf16),
        "finfr": np.concatenate([-Fi, Fr], axis=1).astype(bf16),
        "ident": np.eye(128, dtype=np.float32).astype(bf16),
        "w2": w2,
    }


def _row0_pair_sums(a, p, n, neg_idx):
    """Host-side k1=0 row contributions (unscaled |diff| sums), [B,3] float64."""
    def row0(x):  # x [*,C,H,W] f32 -> normalized row-0 features [*,C,W] complex
        r0 = np.fft.fft(x.sum(axis=-2), axis=-1)
        nrm = np.sqrt((np.abs(r0) ** 2).sum(axis=-2, keepdims=True))
        return r0 / nrm

    f0a, f0p, f0n = row0(a), row0(p), row0(n)
    out = np.zeros((B, 3))
    for s in range(B):
        j1, j2 = int(neg_idx[s, 0]), int(neg_idx[s, 1])
        out[s, 0] = np.abs(f0a[s] - f0p[s]).sum()
        out[s, 1] = np.abs(f0a[s] - f0n[j1]).sum()
        out[s, 2] = np.abs(f0a[s] - f0n[j2]).sum()
    return out


def run_cores(in_maps, trace=False):
    from concourse.bass_utils import run_bass_kernel_spmd

    nc = _get_program()
    return run_bass_kernel_spmd(nc, in_maps, list(range(N_CORES)), trace=trace)


def make_in_maps(a, p, n, neg_idx):
    consts = _const_inputs()
    a16 = a.astype(bf16)
    p16 = p.astype(bf16)
    n16 = n.astype(bf16)
    in_maps = []
    for core in range(N_CORES):
        sl = slice(core * SPC, (core + 1) * SPC)
        idx = neg_idx[sl].reshape(-1).astype(np.int64)
        in_maps.append(
            {
                "a_in": np.ascontiguousarray(a16[sl]),
                "p_in": np.ascontiguousarray(p16[sl]),
                "n_in": np.ascontiguousarray(n16[idx]),
                **consts,
            }
        )
    return in_maps


def finish(results, a, p, n, neg_idx):
    """results: list of per-core dicts with 'rs_out' [128, SPC, 3]."""
    main = np.zeros((B, 3))
    for core in range(N_CORES):
        rs = np.asarray(results[core]["rs_out"], np.float64)  # [128, SPC, 3]
        main[core * SPC:(core + 1) * SPC] = rs.sum(axis=0).reshape(SPC, 3)
    row0 = _row0_pair_sums(a, p, n, neg_idx)
    d = 0.01 * (main + row0) / (C * H * W)  # [B,3] means: ap, an1, an2
    total = (d[:, 0] / (d[:, 1] + 1e-7) + d[:, 0] / (d[:, 2] + 1e-7)).sum()
    return np.float32(total / (K * B))


def kernel(a, p, n, neg_idx):
    a = np.asarray(a, np.float32)
    p = np.asarray(p, np.float32)
    n = np.asarray(n, np.float32)
    neg_idx = np.asarray(neg_idx)
    res = run_cores(make_in_maps(a, p, n, neg_idx))
    return finish(res.results, a, p, n, neg_idx)

